# revision 32
# baseline (speedup 1.0000x reference)
"""GAT layer (nn_GATLayer) Trainium2 Bass kernel.

Data-parallel over batch B=8 across 8 NeuronCores (one batch element per core).

Per core (batch b), with N=2048, D=64:
  Wh   = h @ W.T                         [N, D]
  s_i  = Wh @ a1, s_j = Wh @ a2          [N]
  e    = LeakyReLU_{0.2}(s_i + s_j^T)    [N, N]   (rank-1 structure!)
  alpha = softmax_j(e)                   [N, N]
  h'   = alpha @ Wh                      [N, D]

Key tricks:
  - row max of e is m_i = LRelu(s_i + max_j s_j): O(N), no N^2 pass
    (max commutes with the monotone LeakyReLU over the rank-1 logits).
  - e-chunk built in ONE ACT pass: Prelu(S_J_bcast + bias=s_i_col, alpha=0.2)
    (or two DVE ops on some chunks, to balance ACT/DVE).
  - Exp pass emits row sums for free via accum_out.
  - Exp writes fp16 P directly; alpha = P16*(1/D) in fp32 for HBM, while
    the h' matmul path transposes unnormalized P16 on the PE (fp16 PSUM),
    DVE-evicts to SBUF, runs fp16 matmuls with Wh (fp32 accumulate), and
    applies the 1/D row scaling on the tiny h' tiles at the end.
  - alpha HBM writes split across the Sync HWDGE queue (96 rows) and the
    GPSIMD SWDGE queue (32 rows) to use more DMA engines.
"""

import numpy as np

_B, _N, _DIN, _DOUT = 8, 2048, 64, 64
_NC = 8            # cores
_CH = 128          # rows per chunk
_NCHUNK = _N // _CH  # 16
_GRP = 4           # i-chunks per h' matmul group
_HTP = 8           # j-blocks per PE-transpose round (1 fp16 PSUM bank)
_NGRP = _NCHUNK // _GRP

_DVE_BUILD = frozenset({4, 9, 14})
_DMA_EVICT = False
_WARMERS = False

_cache = {}


def _build():
    import concourse.bacc as bacc
    import concourse.mybir as mybir
    import concourse.tile as tile
    from concourse import masks
    from concourse.bass_interp import get_hw_module

    F = mybir.ActivationFunctionType
    AX = mybir.AxisListType
    f32 = mybir.dt.float32
    f16 = mybir.dt.float16

    nc = bacc.Bacc("TRN2", target_bir_lowering=False, debug=False,
                   num_devices=_NC)
    h_in = nc.dram_tensor("h", [_N, _DIN], f32, kind="ExternalInput")
    w_in = nc.dram_tensor("W", [_DOUT, _DIN], f32, kind="ExternalInput")
    a_in = nc.dram_tensor("a", [2 * _DOUT], f32, kind="ExternalInput")
    alpha_out = nc.dram_tensor("alpha", [_N, _N], f32, kind="ExternalOutput")
    hp_out = nc.dram_tensor("h_prime", [_N, _DOUT], f32, kind="ExternalOutput")

    with tile.TileContext(nc) as tc:
        with tc.tile_pool(name="const", bufs=1) as constp, \
             tc.tile_pool(name="elp", bufs=4) as elp, \
             tc.tile_pool(name="pp", bufs=3) as pp, \
             tc.tile_pool(name="alp", bufs=2) as alp, \
             tc.tile_pool(name="smallp", bufs=8) as smallp, \
             tc.tile_pool(name="rcp", bufs=10) as rcp, \
             tc.tile_pool(name="atg", bufs=2) as atgp, \
             tc.tile_pool(name="hps", bufs=2) as hpsp, \
             tc.tile_pool(name="ps_pro", bufs=2, space="PSUM") as ps_pro, \
             tc.tile_pool(name="ps_one", bufs=1, space="PSUM") as ps_one, \
             tc.tile_pool(name="ps_warm", bufs=1, space="PSUM") as ps_warm, \
             tc.tile_pool(name="ps_tp", bufs=2, space="PSUM") as ps_tp, \
             tc.tile_pool(name="ps_hp", bufs=2, space="PSUM") as ps_hp:

            ident = constp.tile([128, 128], f32)
            masks.make_identity(nc, ident[:])
            ident16 = constp.tile([128, 128], f16)
            nc.vector.tensor_copy(ident16[:], ident[:])

            # ---- loads ----
            hsb = constp.tile([128, _NCHUNK * _DIN], f32)
            hsb_v = hsb[:].rearrange("p (t d) -> p t d", d=_DIN)
            nc.sync.dma_start(hsb_v, h_in.rearrange("(t p) d -> p t d", p=_CH))
            wsb = constp.tile([_DOUT, _DIN], f32)
            nc.sync.dma_start(wsb[:], w_in[:])
            a2 = constp.tile([_DIN, 2], f32)
            nc.sync.dma_start(a2[:], a_in.rearrange("(c d) -> d c", c=2))

            # ---- WT = W.T; V = W^T [a1 a2] ----
            WT = constp.tile([_DIN, _DOUT], f32)
            pwt = ps_pro.tile([128, 512], f32, tag="ps")
            nc.tensor.transpose(pwt[0:_DIN, 0:_DOUT], wsb[:],
                                ident[0:_DOUT, 0:_DOUT])
            nc.scalar.copy(WT[:], pwt[0:_DIN, 0:_DOUT])
            V = constp.tile([_DIN, 2], f32)
            pv = ps_pro.tile([128, 512], f32, tag="ps")
            nc.tensor.matmul(pv[0:_DIN, 0:2], wsb[:], a2[:],
                             start=True, stop=True)
            nc.scalar.copy(V[:], pv[0:_DIN, 0:2])

            # ---- hT rounds interleaved with s_j row + s col matmuls ----
            hT = constp.tile([_DIN, _N], f32)
            sjrow = constp.tile([1, _N], f32)
            scol = constp.tile([128, 2 * _NCHUNK], f32)
            psc = ps_one.tile([128, 512], f32, tag="psc")
            for r in range(4):
                ptp = ps_pro.tile([128, 512], f32, tag="ps")
                for tt in range(4):
                    t = r * 4 + tt
                    nc.tensor.transpose(
                        ptp[0:_DIN, tt * _CH:(tt + 1) * _CH],
                        hsb_v[:, t, :], ident[:])
                nc.vector.tensor_copy(hT[:, r * 512:(r + 1) * 512],
                                      ptp[0:_DIN, :])
                ps2 = ps_pro.tile([128, 512], f32, tag="ps")
                nc.tensor.matmul(ps2[0:1, :], V[:, 1:2],
                                 hT[:, r * 512:(r + 1) * 512],
                                 start=True, stop=True)
                nc.scalar.copy(sjrow[:, r * 512:(r + 1) * 512], ps2[0:1, :])
                for tt in range(4):
                    t = r * 4 + tt
                    nc.tensor.matmul(psc[:, 2 * t:2 * t + 2],
                                     hT[:, t * _CH:(t + 1) * _CH],
                                     V[:], start=True, stop=True)
            nc.vector.tensor_copy(scol[:], psc[:, 0:2 * _NCHUNK])

            # ---- S_J broadcast tile [128, N] (gpsimd) ----
            sj_b = constp.tile([128, _N], f32)
            nc.gpsimd.partition_broadcast(sj_b[:], sjrow[0:1, :])

            # ---- M = max_j s_j via scol (2-stage reduce) ----
            scol_j = scol[:].rearrange("p (t c) -> p t c", c=2)[:, :, 1]
            r1 = smallp.tile([128, 1], f32, tag="r1")
            nc.vector.reduce_max(r1[:], scol_j, axis=AX.X)
            pr1 = ps_pro.tile([128, 512], f32, tag="ps")
            nc.tensor.transpose(pr1[0:1, 0:128], r1[:], ident[:])
            r2 = smallp.tile([1, 128], f32, tag="r2")
            nc.scalar.copy(r2[:], pr1[0:1, 0:128])
            mrow = smallp.tile([1, 1], f32, tag="mrow")
            nc.vector.reduce_max(mrow[:], r2[:], axis=AX.X)
            mcol = constp.tile([128, 1], f32)
            nc.gpsimd.partition_broadcast(mcol[:], mrow[:])

            # ---- negm for all chunks at once [128, 16] ----
            negm = constp.tile([128, _NCHUNK], f32)
            scol_i = scol[:].rearrange("p (t c) -> p t c", c=2)[:, :, 0]
            u_all = smallp.tile([128, _NCHUNK], f32, tag="u_all")
            nc.vector.tensor_scalar_add(u_all[:], scol_i, mcol[:])
            m_all = smallp.tile([128, _NCHUNK], f32, tag="m_all")
            nc.scalar.activation(m_all[:], u_all[:], F.Prelu, alpha=0.2)
            nc.vector.tensor_scalar_mul(negm[:], m_all[:], -1.0)

            # ---- 0.2-prescaled copies for DVE-side el builds ----
            sj02 = constp.tile([128, _N], f32)
            nc.vector.tensor_scalar_mul(sj02[:], sj_b[:], 0.2)
            scol02 = constp.tile([128, 2 * _NCHUNK], f32)
            nc.vector.tensor_scalar_mul(scol02[:], scol[:], 0.2)

            # ---- Wh blocks [128, 64] x 16 in fp16 (batched) ----
            hT16 = constp.tile([_DIN, _N], f16)
            nc.vector.tensor_copy(hT16[:], hT[:])
            WT16 = constp.tile([_DIN, _DOUT], f16)
            nc.vector.tensor_copy(WT16[:], WT[:])
            wh16 = constp.tile([128, _NCHUNK * _DOUT], f16)
            for r in range(2):
                pb = ps_pro.tile([128, 512], f32, tag="ps")
                for tt in range(8):
                    t = r * 8 + tt
                    nc.tensor.matmul(pb[:, tt * _DOUT:(tt + 1) * _DOUT],
                                     hT16[:, t * _CH:(t + 1) * _CH],
                                     WT16[:], start=True, stop=True)
                nc.vector.tensor_copy(wh16[:, r * 512:(r + 1) * 512], pb[:])

            rcs = {}
            # ---- main loop over i-chunk groups ----
            for g in range(_NGRP):
                at = atgp.tile([128, _NCHUNK * _GRP * _CH], f16, tag="at")
                at_v = at[:].rearrange("p (j c) -> p j c", j=_NCHUNK)
                for q in range(_GRP):
                    ci = g * _GRP + q
                    el = elp.tile([128, _N], f32, tag="el")
                    if ci in _DVE_BUILD:
                        u2 = elp.tile([128, _N], f32, tag="u2")
                        nc.vector.tensor_scalar_add(
                            u2[:], sj02[:], scol02[:, 2 * ci:2 * ci + 1])
                        nc.vector.scalar_tensor_tensor(
                            el[:], sj_b[:], scol[:, 2 * ci:2 * ci + 1], u2[:],
                            op0=mybir.AluOpType.add, op1=mybir.AluOpType.max)
                    else:
                        nc.scalar.activation(el[:], sj_b[:], F.Prelu,
                                             bias=scol[:, 2 * ci:2 * ci + 1],
                                             alpha=0.2)
                    P16 = pp.tile([128, _N], f16, tag="P16")
                    D = smallp.tile([128, 1], f32, tag="D")
                    nc.scalar.activation(P16[:], el[:], F.Exp,
                                         bias=negm[:, ci:ci + 1],
                                         accum_out=D[:])
                    rc = rcp.tile([128, 1], f32, tag="rc")
                    nc.vector.reciprocal(rc[:], D[:])
                    rcs[ci] = rc
                    al = alp.tile([128, _N], f32, tag="al")
                    nc.vector.tensor_scalar_mul(al[:], P16[:], rc[:])
                    nc.sync.dma_start(
                        alpha_out[ci * _CH:ci * _CH + 96, :], al[0:96, :])
                    nc.gpsimd.dma_start(
                        alpha_out[ci * _CH + 96:(ci + 1) * _CH, :],
                        al[96:128, :])
                    # fp16 PE transposes of the 16 j-blocks, 2 rounds of 8
                    for r in range(_NCHUNK // _HTP):
                        tp = ps_tp.tile([128, _HTP * _CH], f16, tag="tp")
                        tp_v = tp[:].rearrange("p (j c) -> p j c", j=_HTP)
                        for jj in range(_HTP):
                            j = r * _HTP + jj
                            nc.tensor.transpose(tp_v[:, jj, :],
                                                P16[:, j * _CH:(j + 1) * _CH],
                                                ident16[:])
                        dst = at_v[:, r * _HTP:(r + 1) * _HTP,
                                   q * _CH:(q + 1) * _CH]
                        if _DMA_EVICT:
                            nc.sync.dma_start(dst, tp_v[:, :, :])
                        else:
                            nc.vector.tensor_copy(dst, tp_v[:, :, :])
                        if _WARMERS:
                            pw2 = ps_warm.tile([128, 512], f32, tag="warm")
                            nc.tensor.matmul(pw2[:], hT16[:, 0:128],
                                             hT16[:, 0:512],
                                             start=True, stop=True)
                # h'^T group matmul: [64, GRP*CH], contraction over j
                hpt = ps_hp.tile([_DOUT, _GRP * _CH], f32, tag="hpt")
                for half in range(_GRP * _CH // 512):
                    hs = slice(half * 512, (half + 1) * 512)
                    for j in range(_NCHUNK):
                        nc.tensor.matmul(hpt[:, hs],
                                         wh16[:, j * _DOUT:(j + 1) * _DOUT],
                                         at_v[:, j, hs],
                                         start=(j == 0),
                                         stop=(j == _NCHUNK - 1))
                hpt_sb = hpsp.tile([_DOUT, _GRP * _CH], f32, tag="hpt_sb")
                nc.vector.tensor_copy(hpt_sb[:], hpt[:])
                for q in range(_GRP):
                    php = ps_pro.tile([128, 512], f32, tag="ps")
                    nc.tensor.transpose(php[:, 0:_DOUT],
                                        hpt_sb[:, q * _CH:(q + 1) * _CH],
                                        ident[0:_DOUT, 0:_DOUT])
                    hp_sb = hpsp.tile([_CH, _DOUT], f32, tag="hp_sb")
                    nc.vector.tensor_scalar_mul(hp_sb[:], php[:, 0:_DOUT],
                                                rcs[g * _GRP + q][:])
                    r0 = (g * _GRP + q) * _CH
                    nc.sync.dma_start(hp_out[r0:r0 + _CH, :], hp_sb[:])

    nc.compile()
    nc.m = get_hw_module(nc.m)
    return nc


def kernel(h, W, a):
    from concourse.bass_utils import run_bass_kernel_spmd

    if "nc" not in _cache:
        _cache["nc"] = _build()
    nc = _cache["nc"]

    h = np.ascontiguousarray(np.asarray(h, dtype=np.float32))
    W = np.ascontiguousarray(np.asarray(W, dtype=np.float32))
    a = np.ascontiguousarray(np.asarray(a, dtype=np.float32))

    in_maps = [{"h": h[b], "W": W, "a": a} for b in range(_B)]
    res = run_bass_kernel_spmd(nc, in_maps, list(range(_NC)))
    h_prime = np.stack([res.results[b]["h_prime"] for b in range(_B)])
    alpha = np.stack([res.results[b]["alpha"] for b in range(_B)])
    return h_prime, alpha


# revision 33
# speedup vs baseline: 1.0319x; 1.0319x over previous
"""GAT layer (nn_GATLayer) Trainium2 Bass kernel.

Data-parallel over batch B=8 across 8 NeuronCores (one batch element per core).

Per core (batch b), with N=2048, D=64:
  Wh   = h @ W.T                         [N, D]
  s_i  = Wh @ a1, s_j = Wh @ a2          [N]
  e    = LeakyReLU_{0.2}(s_i + s_j^T)    [N, N]   (rank-1 structure!)
  alpha = softmax_j(e)                   [N, N]
  h'   = alpha @ Wh                      [N, D]

Key tricks:
  - row max of e is m_i = LRelu(s_i + max_j s_j): O(N), no N^2 pass
    (max commutes with the monotone LeakyReLU over the rank-1 logits).
  - e-chunk built in ONE ACT pass: Prelu(S_J_bcast + bias=s_i_col, alpha=0.2)
    (or two DVE ops on some chunks, to balance ACT/DVE).
  - Exp pass emits row sums for free via accum_out.
  - Exp writes fp16 P directly; alpha = P16*(1/D) in fp32 for HBM, while
    the h' matmul path transposes unnormalized P16 on the PE (fp16 PSUM),
    DVE-evicts to SBUF, runs fp16 matmuls with Wh (fp32 accumulate), and
    applies the 1/D row scaling on the tiny h' tiles at the end.
  - alpha HBM writes split across the Sync HWDGE queue (96 rows) and the
    GPSIMD SWDGE queue (32 rows) to use more DMA engines.
"""

import numpy as np

_B, _N, _DIN, _DOUT = 8, 2048, 64, 64
_NC = 8            # cores
_CH = 128          # rows per chunk
_NCHUNK = _N // _CH  # 16
_GRP = 4           # i-chunks per h' matmul group
_HTP = 8           # j-blocks per PE-transpose round (1 fp16 PSUM bank)
_NGRP = _NCHUNK // _GRP

_DVE_BUILD = frozenset({4, 9, 14})
_DMA_EVICT = False
_WARMERS = False

_cache = {}


def _build():
    import concourse.bacc as bacc
    import concourse.mybir as mybir
    import concourse.tile as tile
    from concourse import masks
    from concourse.bass_interp import get_hw_module

    F = mybir.ActivationFunctionType
    AX = mybir.AxisListType
    f32 = mybir.dt.float32
    f16 = mybir.dt.float16

    nc = bacc.Bacc("TRN2", target_bir_lowering=False, debug=False,
                   num_devices=_NC)
    h_in = nc.dram_tensor("h", [_N, _DIN], f32, kind="ExternalInput")
    w_in = nc.dram_tensor("W", [_DOUT, _DIN], f32, kind="ExternalInput")
    a_in = nc.dram_tensor("a", [2 * _DOUT], f32, kind="ExternalInput")
    alpha_out = nc.dram_tensor("alpha", [_N, _N], f32, kind="ExternalOutput")
    hp_out = nc.dram_tensor("h_prime", [_N, _DOUT], f32, kind="ExternalOutput")

    with tile.TileContext(nc) as tc:
        with tc.tile_pool(name="const", bufs=1) as constp, \
             tc.tile_pool(name="elp", bufs=3) as elp, \
             tc.tile_pool(name="pp", bufs=3) as pp, \
             tc.tile_pool(name="alp", bufs=3) as alp, \
             tc.tile_pool(name="smallp", bufs=8) as smallp, \
             tc.tile_pool(name="rcp", bufs=10) as rcp, \
             tc.tile_pool(name="atg", bufs=2) as atgp, \
             tc.tile_pool(name="hps", bufs=2) as hpsp, \
             tc.tile_pool(name="ps_pro", bufs=2, space="PSUM") as ps_pro, \
             tc.tile_pool(name="ps_one", bufs=1, space="PSUM") as ps_one, \
             tc.tile_pool(name="ps_warm", bufs=1, space="PSUM") as ps_warm, \
             tc.tile_pool(name="ps_tp", bufs=2, space="PSUM") as ps_tp, \
             tc.tile_pool(name="ps_hp", bufs=2, space="PSUM") as ps_hp:

            ident = constp.tile([128, 128], f32)
            masks.make_identity(nc, ident[:])
            ident16 = constp.tile([128, 128], f16)
            nc.vector.tensor_copy(ident16[:], ident[:])

            # ---- loads ----
            hsb = constp.tile([128, _NCHUNK * _DIN], f32)
            hsb_v = hsb[:].rearrange("p (t d) -> p t d", d=_DIN)
            nc.sync.dma_start(hsb_v, h_in.rearrange("(t p) d -> p t d", p=_CH))
            wsb = constp.tile([_DOUT, _DIN], f32)
            nc.sync.dma_start(wsb[:], w_in[:])
            a2 = constp.tile([_DIN, 2], f32)
            nc.sync.dma_start(a2[:], a_in.rearrange("(c d) -> d c", c=2))

            # ---- WT = W.T; V = W^T [a1 a2] ----
            WT = constp.tile([_DIN, _DOUT], f32)
            pwt = ps_pro.tile([128, 512], f32, tag="ps")
            nc.tensor.transpose(pwt[0:_DIN, 0:_DOUT], wsb[:],
                                ident[0:_DOUT, 0:_DOUT])
            nc.scalar.copy(WT[:], pwt[0:_DIN, 0:_DOUT])
            V = constp.tile([_DIN, 2], f32)
            pv = ps_pro.tile([128, 512], f32, tag="ps")
            nc.tensor.matmul(pv[0:_DIN, 0:2], wsb[:], a2[:],
                             start=True, stop=True)
            nc.scalar.copy(V[:], pv[0:_DIN, 0:2])

            # ---- hT rounds interleaved with s_j row + s col matmuls ----
            hT = constp.tile([_DIN, _N], f32)
            sjrow = constp.tile([1, _N], f32)
            scol = constp.tile([128, 2 * _NCHUNK], f32)
            psc = ps_one.tile([128, 512], f32, tag="psc")
            for r in range(4):
                ptp = ps_pro.tile([128, 512], f32, tag="ps")
                for tt in range(4):
                    t = r * 4 + tt
                    nc.tensor.transpose(
                        ptp[0:_DIN, tt * _CH:(tt + 1) * _CH],
                        hsb_v[:, t, :], ident[:])
                nc.vector.tensor_copy(hT[:, r * 512:(r + 1) * 512],
                                      ptp[0:_DIN, :])
                ps2 = ps_pro.tile([128, 512], f32, tag="ps")
                nc.tensor.matmul(ps2[0:1, :], V[:, 1:2],
                                 hT[:, r * 512:(r + 1) * 512],
                                 start=True, stop=True)
                nc.scalar.copy(sjrow[:, r * 512:(r + 1) * 512], ps2[0:1, :])
                for tt in range(4):
                    t = r * 4 + tt
                    nc.tensor.matmul(psc[:, 2 * t:2 * t + 2],
                                     hT[:, t * _CH:(t + 1) * _CH],
                                     V[:], start=True, stop=True)
            nc.vector.tensor_copy(scol[:], psc[:, 0:2 * _NCHUNK])

            # ---- S_J broadcast tile [128, N] (gpsimd) ----
            sj_b = constp.tile([128, _N], f32)
            nc.gpsimd.partition_broadcast(sj_b[:], sjrow[0:1, :])

            # ---- M = max_j s_j via scol (2-stage reduce) ----
            scol_j = scol[:].rearrange("p (t c) -> p t c", c=2)[:, :, 1]
            r1 = smallp.tile([128, 1], f32, tag="r1")
            nc.vector.reduce_max(r1[:], scol_j, axis=AX.X)
            pr1 = ps_pro.tile([128, 512], f32, tag="ps")
            nc.tensor.transpose(pr1[0:1, 0:128], r1[:], ident[:])
            r2 = smallp.tile([1, 128], f32, tag="r2")
            nc.scalar.copy(r2[:], pr1[0:1, 0:128])
            mrow = smallp.tile([1, 1], f32, tag="mrow")
            nc.vector.reduce_max(mrow[:], r2[:], axis=AX.X)
            mcol = constp.tile([128, 1], f32)
            nc.gpsimd.partition_broadcast(mcol[:], mrow[:])

            # ---- negm for all chunks at once [128, 16] ----
            negm = constp.tile([128, _NCHUNK], f32)
            scol_i = scol[:].rearrange("p (t c) -> p t c", c=2)[:, :, 0]
            u_all = smallp.tile([128, _NCHUNK], f32, tag="u_all")
            nc.vector.tensor_scalar_add(u_all[:], scol_i, mcol[:])
            m_all = smallp.tile([128, _NCHUNK], f32, tag="m_all")
            nc.scalar.activation(m_all[:], u_all[:], F.Prelu, alpha=0.2)
            nc.vector.tensor_scalar_mul(negm[:], m_all[:], -1.0)

            # ---- 0.2-prescaled copies for DVE-side el builds ----
            sj02 = constp.tile([128, _N], f32)
            nc.vector.tensor_scalar_mul(sj02[:], sj_b[:], 0.2)
            scol02 = constp.tile([128, 2 * _NCHUNK], f32)
            nc.vector.tensor_scalar_mul(scol02[:], scol[:], 0.2)

            # ---- Wh blocks [128, 64] x 16 in fp16 (batched) ----
            hT16 = constp.tile([_DIN, _N], f16)
            nc.vector.tensor_copy(hT16[:], hT[:])
            WT16 = constp.tile([_DIN, _DOUT], f16)
            nc.vector.tensor_copy(WT16[:], WT[:])
            wh16 = constp.tile([128, _NCHUNK * _DOUT], f16)
            for r in range(2):
                pb = ps_pro.tile([128, 512], f32, tag="ps")
                for tt in range(8):
                    t = r * 8 + tt
                    nc.tensor.matmul(pb[:, tt * _DOUT:(tt + 1) * _DOUT],
                                     hT16[:, t * _CH:(t + 1) * _CH],
                                     WT16[:], start=True, stop=True)
                nc.vector.tensor_copy(wh16[:, r * 512:(r + 1) * 512], pb[:])

            rcs = {}
            # ---- main loop over i-chunk groups ----
            for g in range(_NGRP):
                at = atgp.tile([128, _NCHUNK * _GRP * _CH], f16, tag="at")
                at_v = at[:].rearrange("p (j c) -> p j c", j=_NCHUNK)
                for q in range(_GRP):
                    ci = g * _GRP + q
                    el = elp.tile([128, _N], f32, tag="el")
                    if ci in _DVE_BUILD:
                        u2 = elp.tile([128, _N], f32, tag="u2")
                        nc.vector.tensor_scalar_add(
                            u2[:], sj02[:], scol02[:, 2 * ci:2 * ci + 1])
                        nc.vector.scalar_tensor_tensor(
                            el[:], sj_b[:], scol[:, 2 * ci:2 * ci + 1], u2[:],
                            op0=mybir.AluOpType.add, op1=mybir.AluOpType.max)
                    else:
                        nc.scalar.activation(el[:], sj_b[:], F.Prelu,
                                             bias=scol[:, 2 * ci:2 * ci + 1],
                                             alpha=0.2)
                    P16 = pp.tile([128, _N], f16, tag="P16")
                    D = smallp.tile([128, 1], f32, tag="D")
                    nc.scalar.activation(P16[:], el[:], F.Exp,
                                         bias=negm[:, ci:ci + 1],
                                         accum_out=D[:])
                    rc = rcp.tile([128, 1], f32, tag="rc")
                    nc.vector.reciprocal(rc[:], D[:])
                    rcs[ci] = rc
                    al = alp.tile([128, _N], f32, tag="al")
                    nc.vector.tensor_scalar_mul(al[:], P16[:], rc[:])
                    nc.sync.dma_start(
                        alpha_out[ci * _CH:ci * _CH + 96, :], al[0:96, :])
                    nc.gpsimd.dma_start(
                        alpha_out[ci * _CH + 96:(ci + 1) * _CH, :],
                        al[96:128, :])
                    # fp16 PE transposes of the 16 j-blocks, 2 rounds of 8
                    for r in range(_NCHUNK // _HTP):
                        tp = ps_tp.tile([128, _HTP * _CH], f16, tag="tp")
                        tp_v = tp[:].rearrange("p (j c) -> p j c", j=_HTP)
                        for jj in range(_HTP):
                            j = r * _HTP + jj
                            nc.tensor.transpose(tp_v[:, jj, :],
                                                P16[:, j * _CH:(j + 1) * _CH],
                                                ident16[:])
                        dst = at_v[:, r * _HTP:(r + 1) * _HTP,
                                   q * _CH:(q + 1) * _CH]
                        if _DMA_EVICT:
                            nc.sync.dma_start(dst, tp_v[:, :, :])
                        else:
                            nc.vector.tensor_copy(dst, tp_v[:, :, :])
                        if _WARMERS:
                            pw2 = ps_warm.tile([128, 512], f32, tag="warm")
                            nc.tensor.matmul(pw2[:], hT16[:, 0:128],
                                             hT16[:, 0:512],
                                             start=True, stop=True)
                # h'^T group matmul: [64, GRP*CH], contraction over j
                hpt = ps_hp.tile([_DOUT, _GRP * _CH], f32, tag="hpt")
                for half in range(_GRP * _CH // 512):
                    hs = slice(half * 512, (half + 1) * 512)
                    for j in range(_NCHUNK):
                        nc.tensor.matmul(hpt[:, hs],
                                         wh16[:, j * _DOUT:(j + 1) * _DOUT],
                                         at_v[:, j, hs],
                                         start=(j == 0),
                                         stop=(j == _NCHUNK - 1))
                hpt_sb = hpsp.tile([_DOUT, _GRP * _CH], f32, tag="hpt_sb")
                nc.vector.tensor_copy(hpt_sb[:], hpt[:])
                for q in range(_GRP):
                    php = ps_pro.tile([128, 512], f32, tag="ps")
                    nc.tensor.transpose(php[:, 0:_DOUT],
                                        hpt_sb[:, q * _CH:(q + 1) * _CH],
                                        ident[0:_DOUT, 0:_DOUT])
                    hp_sb = hpsp.tile([_CH, _DOUT], f32, tag="hp_sb")
                    nc.vector.tensor_scalar_mul(hp_sb[:], php[:, 0:_DOUT],
                                                rcs[g * _GRP + q][:])
                    r0 = (g * _GRP + q) * _CH
                    nc.sync.dma_start(hp_out[r0:r0 + _CH, :], hp_sb[:])

    nc.compile()
    nc.m = get_hw_module(nc.m)
    return nc


def kernel(h, W, a):
    from concourse.bass_utils import run_bass_kernel_spmd

    if "nc" not in _cache:
        _cache["nc"] = _build()
    nc = _cache["nc"]

    h = np.ascontiguousarray(np.asarray(h, dtype=np.float32))
    W = np.ascontiguousarray(np.asarray(W, dtype=np.float32))
    a = np.ascontiguousarray(np.asarray(a, dtype=np.float32))

    in_maps = [{"h": h[b], "W": W, "a": a} for b in range(_B)]
    res = run_bass_kernel_spmd(nc, in_maps, list(range(_NC)))
    h_prime = np.stack([res.results[b]["h_prime"] for b in range(_B)])
    alpha = np.stack([res.results[b]["alpha"] for b in range(_B)])
    return h_prime, alpha


# revision 34
# speedup vs baseline: 1.0426x; 1.0104x over previous
"""GAT layer (nn_GATLayer) Trainium2 Bass kernel.

Data-parallel over batch B=8 across 8 NeuronCores (one batch element per core).

Per core (batch b), with N=2048, D=64:
  Wh   = h @ W.T                         [N, D]
  s_i  = Wh @ a1, s_j = Wh @ a2          [N]
  e    = LeakyReLU_{0.2}(s_i + s_j^T)    [N, N]   (rank-1 structure!)
  alpha = softmax_j(e)                   [N, N]
  h'   = alpha @ Wh                      [N, D]

Key tricks:
  - row max of e is m_i = LRelu(s_i + max_j s_j): O(N), no N^2 pass
    (max commutes with the monotone LeakyReLU over the rank-1 logits).
  - e-chunk built in ONE ACT pass: Prelu(S_J_bcast + bias=s_i_col, alpha=0.2)
    (or two DVE ops on some chunks, to balance ACT/DVE).
  - Exp pass emits row sums for free via accum_out.
  - Exp writes fp16 P directly; alpha = P16*(1/D) in fp32 for HBM, while
    the h' matmul path transposes unnormalized P16 on the PE (fp16 PSUM),
    DVE-evicts to SBUF, runs fp16 matmuls with Wh (fp32 accumulate), and
    applies the 1/D row scaling on the tiny h' tiles at the end.
  - alpha HBM writes split across the Sync HWDGE queue (96 rows) and the
    GPSIMD SWDGE queue (32 rows) to use more DMA engines.
"""

import numpy as np

_B, _N, _DIN, _DOUT = 8, 2048, 64, 64
_NC = 8            # cores
_CH = 128          # rows per chunk
_NCHUNK = _N // _CH  # 16
_GRP = 4           # i-chunks per h' matmul group
_HTP = 8           # j-blocks per PE-transpose round (1 fp16 PSUM bank)
_NGRP = _NCHUNK // _GRP

_DVE_BUILD = frozenset({4, 9, 14})
_DMA_EVICT = False
_WARMERS = False

_cache = {}


def _build():
    import concourse.bacc as bacc
    import concourse.mybir as mybir
    import concourse.tile as tile
    from concourse import masks
    from concourse.bass_interp import get_hw_module

    F = mybir.ActivationFunctionType
    AX = mybir.AxisListType
    f32 = mybir.dt.float32
    f16 = mybir.dt.float16

    nc = bacc.Bacc("TRN2", target_bir_lowering=False, debug=False,
                   num_devices=_NC)
    h_in = nc.dram_tensor("h", [_N, _DIN], f32, kind="ExternalInput")
    w_in = nc.dram_tensor("W", [_DOUT, _DIN], f32, kind="ExternalInput")
    a_in = nc.dram_tensor("a", [2 * _DOUT], f32, kind="ExternalInput")
    alpha_out = nc.dram_tensor("alpha", [_N, _N], f32, kind="ExternalOutput")
    hp_out = nc.dram_tensor("h_prime", [_N, _DOUT], f32, kind="ExternalOutput")

    with tile.TileContext(nc) as tc:
        with tc.tile_pool(name="const", bufs=1) as constp, \
             tc.tile_pool(name="elp", bufs=3) as elp, \
             tc.tile_pool(name="pp", bufs=3) as pp, \
             tc.tile_pool(name="alp", bufs=3) as alp, \
             tc.tile_pool(name="smallp", bufs=8) as smallp, \
             tc.tile_pool(name="rcp", bufs=10) as rcp, \
             tc.tile_pool(name="atg", bufs=2) as atgp, \
             tc.tile_pool(name="hps", bufs=2) as hpsp, \
             tc.tile_pool(name="ps_pro", bufs=3, space="PSUM") as ps_pro, \
             tc.tile_pool(name="ps_one", bufs=1, space="PSUM") as ps_one, \
             tc.tile_pool(name="ps_tp", bufs=2, space="PSUM") as ps_tp, \
             tc.tile_pool(name="ps_hp", bufs=2, space="PSUM") as ps_hp:

            ident = constp.tile([128, 128], f32)
            masks.make_identity(nc, ident[:])
            ident16 = constp.tile([128, 128], f16)
            nc.vector.tensor_copy(ident16[:], ident[:])

            # ---- loads ----
            hsb = constp.tile([128, _NCHUNK * _DIN], f32)
            hsb_v = hsb[:].rearrange("p (t d) -> p t d", d=_DIN)
            nc.sync.dma_start(hsb_v, h_in.rearrange("(t p) d -> p t d", p=_CH))
            wsb = constp.tile([_DOUT, _DIN], f32)
            nc.sync.dma_start(wsb[:], w_in[:])
            a2 = constp.tile([_DIN, 2], f32)
            nc.sync.dma_start(a2[:], a_in.rearrange("(c d) -> d c", c=2))

            # ---- WT = W.T; V = W^T [a1 a2] ----
            WT = constp.tile([_DIN, _DOUT], f32)
            pwt = ps_pro.tile([128, 512], f32, tag="ps")
            nc.tensor.transpose(pwt[0:_DIN, 0:_DOUT], wsb[:],
                                ident[0:_DOUT, 0:_DOUT])
            nc.scalar.copy(WT[:], pwt[0:_DIN, 0:_DOUT])
            V = constp.tile([_DIN, 2], f32)
            pv = ps_pro.tile([128, 512], f32, tag="ps")
            nc.tensor.matmul(pv[0:_DIN, 0:2], wsb[:], a2[:],
                             start=True, stop=True)
            nc.scalar.copy(V[:], pv[0:_DIN, 0:2])

            # ---- hT rounds interleaved with s_j row + s col matmuls ----
            hT = constp.tile([_DIN, _N], f32)
            sjrow = constp.tile([1, _N], f32)
            sj_b = constp.tile([128, _N], f32)
            scol = constp.tile([128, 2 * _NCHUNK], f32)
            psc = ps_one.tile([128, 512], f32, tag="psc")
            for r in range(4):
                ptp = ps_pro.tile([128, 512], f32, tag="ps")
                for tt in range(4):
                    t = r * 4 + tt
                    nc.tensor.transpose(
                        ptp[0:_DIN, tt * _CH:(tt + 1) * _CH],
                        hsb_v[:, t, :], ident[:])
                nc.vector.tensor_copy(hT[:, r * 512:(r + 1) * 512],
                                      ptp[0:_DIN, :])
                ps2 = ps_pro.tile([128, 512], f32, tag="ps")
                nc.tensor.matmul(ps2[0:1, :], V[:, 1:2],
                                 hT[:, r * 512:(r + 1) * 512],
                                 start=True, stop=True)
                nc.scalar.copy(sjrow[:, r * 512:(r + 1) * 512], ps2[0:1, :])
                nc.gpsimd.partition_broadcast(
                    sj_b[:, r * 512:(r + 1) * 512],
                    sjrow[0:1, r * 512:(r + 1) * 512])
                for tt in range(4):
                    t = r * 4 + tt
                    nc.tensor.matmul(psc[:, 2 * t:2 * t + 2],
                                     hT[:, t * _CH:(t + 1) * _CH],
                                     V[:], start=True, stop=True)
            nc.vector.tensor_copy(scol[:], psc[:, 0:2 * _NCHUNK])


            # ---- M = max_j s_j via scol (2-stage reduce) ----
            scol_j = scol[:].rearrange("p (t c) -> p t c", c=2)[:, :, 1]
            r1 = smallp.tile([128, 1], f32, tag="r1")
            nc.vector.reduce_max(r1[:], scol_j, axis=AX.X)
            pr1 = ps_pro.tile([128, 512], f32, tag="ps")
            nc.tensor.transpose(pr1[0:1, 0:128], r1[:], ident[:])
            r2 = smallp.tile([1, 128], f32, tag="r2")
            nc.scalar.copy(r2[:], pr1[0:1, 0:128])
            mrow = smallp.tile([1, 1], f32, tag="mrow")
            nc.vector.reduce_max(mrow[:], r2[:], axis=AX.X)
            mcol = constp.tile([128, 1], f32)
            nc.gpsimd.partition_broadcast(mcol[:], mrow[:])

            # ---- negm for all chunks at once [128, 16] ----
            negm = constp.tile([128, _NCHUNK], f32)
            scol_i = scol[:].rearrange("p (t c) -> p t c", c=2)[:, :, 0]
            u_all = smallp.tile([128, _NCHUNK], f32, tag="u_all")
            nc.vector.tensor_scalar_add(u_all[:], scol_i, mcol[:])
            m_all = smallp.tile([128, _NCHUNK], f32, tag="m_all")
            nc.scalar.activation(m_all[:], u_all[:], F.Prelu, alpha=0.2)
            nc.vector.tensor_scalar_mul(negm[:], m_all[:], -1.0)

            # ---- 0.2-prescaled copies for DVE-side el builds ----
            sj02 = constp.tile([128, _N], f32)
            nc.vector.tensor_scalar_mul(sj02[:], sj_b[:], 0.2)
            scol02 = constp.tile([128, 2 * _NCHUNK], f32)
            nc.vector.tensor_scalar_mul(scol02[:], scol[:], 0.2)

            # ---- Wh blocks [128, 64] x 16 in fp16 (batched) ----
            hT16 = constp.tile([_DIN, _N], f16)
            nc.vector.tensor_copy(hT16[:], hT[:])
            WT16 = constp.tile([_DIN, _DOUT], f16)
            nc.vector.tensor_copy(WT16[:], WT[:])
            wh16 = constp.tile([128, _NCHUNK * _DOUT], f16)
            for r in range(2):
                pb = ps_pro.tile([128, 512], f32, tag="ps")
                for tt in range(8):
                    t = r * 8 + tt
                    nc.tensor.matmul(pb[:, tt * _DOUT:(tt + 1) * _DOUT],
                                     hT16[:, t * _CH:(t + 1) * _CH],
                                     WT16[:], start=True, stop=True)
                nc.vector.tensor_copy(wh16[:, r * 512:(r + 1) * 512], pb[:])

            rcs = {}
            # ---- main loop over i-chunk groups ----
            for g in range(_NGRP):
                at = atgp.tile([128, _NCHUNK * _GRP * _CH], f16, tag="at")
                at_v = at[:].rearrange("p (j c) -> p j c", j=_NCHUNK)
                for q in range(_GRP):
                    ci = g * _GRP + q
                    el = elp.tile([128, _N], f32, tag="el")
                    if ci in _DVE_BUILD:
                        u2 = elp.tile([128, _N], f32, tag="u2")
                        nc.vector.tensor_scalar_add(
                            u2[:], sj02[:], scol02[:, 2 * ci:2 * ci + 1])
                        nc.vector.scalar_tensor_tensor(
                            el[:], sj_b[:], scol[:, 2 * ci:2 * ci + 1], u2[:],
                            op0=mybir.AluOpType.add, op1=mybir.AluOpType.max)
                    else:
                        nc.scalar.activation(el[:], sj_b[:], F.Prelu,
                                             bias=scol[:, 2 * ci:2 * ci + 1],
                                             alpha=0.2)
                    P16 = pp.tile([128, _N], f16, tag="P16")
                    D = smallp.tile([128, 1], f32, tag="D")
                    nc.scalar.activation(P16[:], el[:], F.Exp,
                                         bias=negm[:, ci:ci + 1],
                                         accum_out=D[:])
                    rc = rcp.tile([128, 1], f32, tag="rc")
                    nc.vector.reciprocal(rc[:], D[:])
                    rcs[ci] = rc
                    al = alp.tile([128, _N], f32, tag="al")
                    nc.vector.tensor_scalar_mul(al[:], P16[:], rc[:])
                    nc.sync.dma_start(
                        alpha_out[ci * _CH:ci * _CH + 96, :], al[0:96, :])
                    nc.gpsimd.dma_start(
                        alpha_out[ci * _CH + 96:(ci + 1) * _CH, :],
                        al[96:128, :])
                    # fp16 PE transposes of the 16 j-blocks, 2 rounds of 8
                    for r in range(_NCHUNK // _HTP):
                        tp = ps_tp.tile([128, _HTP * _CH], f16, tag="tp")
                        tp_v = tp[:].rearrange("p (j c) -> p j c", j=_HTP)
                        for jj in range(_HTP):
                            j = r * _HTP + jj
                            nc.tensor.transpose(tp_v[:, jj, :],
                                                P16[:, j * _CH:(j + 1) * _CH],
                                                ident16[:])
                        dst = at_v[:, r * _HTP:(r + 1) * _HTP,
                                   q * _CH:(q + 1) * _CH]
                        if _DMA_EVICT:
                            nc.sync.dma_start(dst, tp_v[:, :, :])
                        else:
                            nc.vector.tensor_copy(dst, tp_v[:, :, :])
                        if _WARMERS:
                            pw2 = ps_warm.tile([128, 512], f32, tag="warm")
                            nc.tensor.matmul(pw2[:], hT16[:, 0:128],
                                             hT16[:, 0:512],
                                             start=True, stop=True)
                # h'^T group matmul: [64, GRP*CH], contraction over j
                hpt = ps_hp.tile([_DOUT, _GRP * _CH], f32, tag="hpt")
                for half in range(_GRP * _CH // 512):
                    hs = slice(half * 512, (half + 1) * 512)
                    for j in range(_NCHUNK):
                        nc.tensor.matmul(hpt[:, hs],
                                         wh16[:, j * _DOUT:(j + 1) * _DOUT],
                                         at_v[:, j, hs],
                                         start=(j == 0),
                                         stop=(j == _NCHUNK - 1))
                hpt_sb = hpsp.tile([_DOUT, _GRP * _CH], f32, tag="hpt_sb")
                nc.vector.tensor_copy(hpt_sb[:], hpt[:])
                for q in range(_GRP):
                    php = ps_pro.tile([128, 512], f32, tag="ps")
                    nc.tensor.transpose(php[:, 0:_DOUT],
                                        hpt_sb[:, q * _CH:(q + 1) * _CH],
                                        ident[0:_DOUT, 0:_DOUT])
                    hp_sb = hpsp.tile([_CH, _DOUT], f32, tag="hp_sb")
                    nc.vector.tensor_scalar_mul(hp_sb[:], php[:, 0:_DOUT],
                                                rcs[g * _GRP + q][:])
                    r0 = (g * _GRP + q) * _CH
                    nc.sync.dma_start(hp_out[r0:r0 + _CH, :], hp_sb[:])

    nc.compile()
    nc.m = get_hw_module(nc.m)
    return nc


def kernel(h, W, a):
    from concourse.bass_utils import run_bass_kernel_spmd

    if "nc" not in _cache:
        _cache["nc"] = _build()
    nc = _cache["nc"]

    h = np.ascontiguousarray(np.asarray(h, dtype=np.float32))
    W = np.ascontiguousarray(np.asarray(W, dtype=np.float32))
    a = np.ascontiguousarray(np.asarray(a, dtype=np.float32))

    in_maps = [{"h": h[b], "W": W, "a": a} for b in range(_B)]
    res = run_bass_kernel_spmd(nc, in_maps, list(range(_NC)))
    h_prime = np.stack([res.results[b]["h_prime"] for b in range(_B)])
    alpha = np.stack([res.results[b]["alpha"] for b in range(_B)])
    return h_prime, alpha
